# revision 1
# baseline (speedup 1.0000x reference)
"""Trainium2 Bass kernel for nn_Contraction_627065225897 (MACE-style symmetric
contraction with per-element (MoE-routed) weights).

Math (per atom n with element e = sorted_indices[n], channel f):
  out[n,f] = sum_p W3[e,p,f] * T3[n,f,p]  +  sum_q W2[e,q,f] * T2[n,f,q]
           + W1[e,0,f] * T1[n,f]
  T3[n,f,p]   = sum_{m1,m2,k} U3[p,m1,m2,k] x_m1 x_m2 x_k
  T2[n,f,q]   = sum_{a,b}     U2[q,a,b]     x_a  x_b
  T1[n,f]     = sum_l         U1[0,l]       x_l
(Equivalent to the reference's Horner evaluation; verified in fp64/fp32.)

Device strategy (per core, 16 atoms, f=128 on partitions):
  - xx[f, (m2,k)] outer products via one DVE op (stride-0 APs)
  - PE transposes xx -> xxT (contraction dim on partitions)
  - PE matmul: T3h[f, (p,m1)+(q)] = xxT.T @ U3m  (moving operand shared)
  - DVE fused multiply-reduce collapses (p,m1) with W3*x weights
Sharding: data-parallel over atoms, 16 atoms/core on 8 cores; per-element
weights are gathered host-side by sorted_indices (routing) and replicated.
"""

import os
import sys
from contextlib import ExitStack

import numpy as np

if "/opt/trn_rl_repo" not in sys.path:
    sys.path.insert(0, "/opt/trn_rl_repo")

B, F, L = 128, 128, 16
E = 10
P3, P2, P1 = 23, 4, 1
NCORES = 8
BS = B // NCORES  # atoms per core = 16
NPM = P3 * L  # 368 = (p, m1) columns, p-major is NOT used; m1-minor within p
NCOL = NPM + P2  # 372 total moving columns (cubic + quadratic)

_CACHE = {}


def _build_program(use_f32r: bool):
    import concourse.bass as bass
    import concourse.mybir as mybir
    import concourse.tile as tile
    from concourse import bacc

    dt = mybir.dt.float32
    nc = bacc.Bacc("TRN2", target_bir_lowering=False, debug=False)

    xs_d = nc.dram_tensor("xs", [128, BS * L], dt, kind="ExternalInput")
    mov_d = nc.dram_tensor("mov", [2, 128, NCOL], dt, kind="ExternalInput")
    cw3_d = nc.dram_tensor("cw3", [128, BS * P3], dt, kind="ExternalInput")
    w2s_d = nc.dram_tensor("w2s", [128, BS * P2], dt, kind="ExternalInput")
    w1s_d = nc.dram_tensor("w1s", [128, BS], dt, kind="ExternalInput")
    u1b_d = nc.dram_tensor("u1b", [128, L], dt, kind="ExternalInput")
    id_d = nc.dram_tensor("ident", [128, 128], dt, kind="ExternalInput")
    out_d = nc.dram_tensor("outT", [128, BS], dt, kind="ExternalOutput")

    mult = mybir.AluOpType.mult
    add = mybir.AluOpType.add

    with tile.TileContext(nc) as tc, ExitStack() as ctx:
        const = ctx.enter_context(tc.tile_pool(name="const", bufs=1))
        work = ctx.enter_context(tc.tile_pool(name="work", bufs=4))
        ps_xx = ctx.enter_context(
            tc.tile_pool(name="ps_xx", bufs=3, space=bass.MemorySpace.PSUM)
        )
        ps_t3 = ctx.enter_context(
            tc.tile_pool(name="ps_t3", bufs=5, space=bass.MemorySpace.PSUM)
        )

        XS = const.tile([128, BS * L], dt)
        nc.sync.dma_start(XS[:], xs_d.ap())
        MOV0 = const.tile([128, NCOL], dt)
        nc.sync.dma_start(MOV0[:], mov_d.ap()[0])
        MOV1 = const.tile([128, NCOL], dt)
        nc.sync.dma_start(MOV1[:], mov_d.ap()[1])
        CW3 = const.tile([128, BS * P3], dt)
        nc.sync.dma_start(CW3[:], cw3_d.ap())
        W2S = const.tile([128, BS * P2], dt)
        nc.sync.dma_start(W2S[:], w2s_d.ap())
        W1S = const.tile([128, BS], dt)
        nc.sync.dma_start(W1S[:], w1s_d.ap())
        U1B = const.tile([128, L], dt)
        nc.sync.dma_start(U1B[:], u1b_d.ap())
        IDENT = const.tile([128, 128], dt)
        nc.sync.dma_start(IDENT[:], id_d.ap())

        # xx[f, (n, m2, k)] = x[f, n, m2] * x[f, n, k] — split into quarters
        # so PE transposes start as soon as the first quarter is ready
        XX = const.tile([128, BS * L * L], dt)
        Q = 4
        for qi in range(Q):
            nq = BS // Q
            xs3 = XS[:, qi * nq * L : (qi + 1) * nq * L].rearrange(
                "p (n a) -> p n a", n=nq
            )
            in0 = xs3.unsqueeze(3).broadcast_to((128, nq, L, L))  # x_m2
            in1 = xs3.unsqueeze(2).broadcast_to((128, nq, L, L))  # x_k
            xxv = XX[:, qi * nq * 256 : (qi + 1) * nq * 256].rearrange(
                "p (n a b) -> p n a b", n=nq, a=L
            )
            nc.vector.tensor_tensor(xxv, in0, in1, op=mult)

        OUT = const.tile([128, BS], dt)
        ACC = const.tile([128, BS * 3], dt)  # per atom: [lin, quad, cubic]
        XXS = const.tile([128, BS * 256], dt)  # all atoms' transposed xx
        W3X = const.tile([128, BS * NPM], dt)  # all atoms' W3*x weights

        maybe_r = (
            (lambda ap: ap.bitcast(mybir.dt.float32r)) if use_f32r else (lambda ap: ap)
        )

        # Phase A: PE transposes + ACT evacuation; GPSIMD builds w3x in parallel
        for n in range(BS):
            xxp = ps_xx.tile([128, 256], dt, tag="xxp")
            nc.tensor.transpose(
                xxp[:, 0:128], XX[:, n * 256 : n * 256 + 128], IDENT[:]
            )
            nc.tensor.transpose(
                xxp[:, 128:256], XX[:, n * 256 + 128 : n * 256 + 256], IDENT[:]
            )
            nc.scalar.copy(XXS[:, n * 256 : (n + 1) * 256], xxp[:])

            # w3x[f, (p, m1)] = W3[e_n, p, f] * x[f, m1]  (GPSIMD, off DVE)
            c0 = CW3[:, n * P3 : (n + 1) * P3].unsqueeze(2).broadcast_to((128, P3, L))
            x0 = XS[:, n * L : (n + 1) * L].unsqueeze(1).broadcast_to((128, P3, L))
            nc.gpsimd.tensor_tensor(
                W3X[:, n * NPM : (n + 1) * NPM].rearrange(
                    "p (a b) -> p a b", a=P3
                ),
                c0,
                x0,
                op=mult,
            )
            # linear: sum_l x_l * W1 * U1_l (DVE, independent of PE)
            sc16 = work.tile([128, L], dt, tag="sc16")
            nc.vector.scalar_tensor_tensor(
                out=sc16[:],
                in0=XS[:, n * L : (n + 1) * L],
                scalar=W1S[:, n : n + 1],
                in1=U1B[:],
                op0=mult,
                op1=mult,
                accum_out=ACC[:, 3 * n : 3 * n + 1],
            )

        # Phase B: PE matmuls + DVE reductions chasing them
        for n in range(BS):
            t3 = ps_t3.tile([128, NCOL], dt, tag="t3")
            nc.tensor.matmul(
                t3[:],
                maybe_r(XXS[:, n * 256 : n * 256 + 128]),
                maybe_r(MOV0[:]),
                start=True,
                stop=False,
            )
            nc.tensor.matmul(
                t3[:],
                maybe_r(XXS[:, n * 256 + 128 : n * 256 + 256]),
                maybe_r(MOV1[:]),
                start=False,
                stop=True,
            )
            # quadratic: sum_q T2_q * W2_q
            sc4 = work.tile([128, P2], dt, tag="sc4")
            nc.vector.scalar_tensor_tensor(
                out=sc4[:],
                in0=t3[:, NPM:NCOL],
                scalar=1.0,
                in1=W2S[:, n * P2 : (n + 1) * P2],
                op0=mult,
                op1=mult,
                accum_out=ACC[:, 3 * n + 1 : 3 * n + 2],
            )
            # cubic: sum_{p,m1} T3 * w3x
            sc368 = work.tile([128, NPM], dt, tag="sc368")
            nc.vector.scalar_tensor_tensor(
                out=sc368[:],
                in0=t3[:, 0:NPM],
                scalar=1.0,
                in1=W3X[:, n * NPM : (n + 1) * NPM],
                op0=mult,
                op1=mult,
                accum_out=ACC[:, 3 * n + 2 : 3 * n + 3],
            )

        nc.vector.tensor_reduce(
            OUT[:],
            ACC[:].rearrange("p (n c) -> p n c", n=BS),
            axis=mybir.AxisListType.X,
            op=add,
        )
        nc.sync.dma_start(out_d.ap(), OUT[:])

    nc.compile()
    return nc


def _host_prep(x, sorted_indices, weights_max, w2, w1, U3, U2, U1):
    """Build per-core input maps (pure layout/gather work)."""
    x = np.ascontiguousarray(x, dtype=np.float32)
    si = np.asarray(sorted_indices).astype(np.int64)
    W3 = np.asarray(weights_max, dtype=np.float32)
    W2 = np.asarray(w2, dtype=np.float32)
    W1 = np.asarray(w1, dtype=np.float32)
    U3 = np.asarray(U3, dtype=np.float32)
    U2 = np.asarray(U2, dtype=np.float32)
    U1 = np.asarray(U1, dtype=np.float32)

    U3r = U3.reshape(P3, L, L, L)  # [p, m1, m2, k]
    U3m = np.ascontiguousarray(U3r.transpose(2, 3, 0, 1).reshape(L * L, P3 * L))
    U2m = np.ascontiguousarray(U2.reshape(P2, L * L).T)  # [(a,b), q]
    mov = np.concatenate([U3m, U2m], axis=1).reshape(2, 128, NCOL)
    mov = np.ascontiguousarray(mov)

    u1b = np.ascontiguousarray(np.tile(U1.reshape(1, L), (128, 1)))
    ident = np.eye(128, dtype=np.float32)

    in_maps = []
    for c in range(NCORES):
        sl = slice(c * BS, (c + 1) * BS)
        sic = si[sl]
        xs = np.ascontiguousarray(x[sl].transpose(1, 0, 2).reshape(128, BS * L))
        cw3 = np.ascontiguousarray(
            W3[sic].transpose(2, 0, 1).reshape(128, BS * P3)
        )
        w2s = np.ascontiguousarray(W2[sic].transpose(2, 0, 1).reshape(128, BS * P2))
        w1s = np.ascontiguousarray(W1[sic][:, 0, :].T)
        in_maps.append(
            {
                "xs": xs,
                "mov": mov,
                "cw3": cw3,
                "w2s": w2s,
                "w1s": w1s,
                "u1b": u1b,
                "ident": ident,
            }
        )
    return in_maps


def _get_nc():
    use_f32r = os.environ.get("KERNEL_F32R", "0") == "1"
    key = ("nc", use_f32r)
    if key not in _CACHE:
        _CACHE[key] = _build_program(use_f32r)
    return _CACHE[key]


def kernel(
    x,
    bincount,
    sorted_indices,
    weights_max,
    w2,
    w1,
    U3,
    U2,
    U1,
    _trace=False,
):
    from concourse.bass_utils import run_bass_kernel_spmd

    nc = _get_nc()
    in_maps = _host_prep(x, sorted_indices, weights_max, w2, w1, U3, U2, U1)
    res = run_bass_kernel_spmd(
        nc, in_maps, core_ids=list(range(NCORES)), trace=_trace
    )
    outs = [res.results[c]["outT"] for c in range(NCORES)]  # each [128f, 16n]
    full = np.concatenate([o.T for o in outs], axis=0)  # [128, 128]
    out = np.ascontiguousarray(full, dtype=np.float32)
    if _trace:
        return out, res
    return out



# revision 3
# speedup vs baseline: 1.9223x; 1.9223x over previous
"""Trainium2 Bass kernel for nn_Contraction_627065225897 (MACE-style symmetric
contraction with per-element (MoE-routed) weights).

Math (per atom n with element e = sorted_indices[n], channel f):
  out[n,f] = sum_{i,j,k} U3[p,i,j,k] W3[e,p,f] x_i x_j x_k
           + sum_{a,b}   U2[q,a,b]   W2[e,q,f] x_a x_b
           + sum_l       U1[l]       W1[e,f]   x_l

Device strategy (per core, 16 atoms, f=128 channels):
  The cubic sum is recast over monomial multisets {i,j,k}:
   - triples with >=2 distinct values: split into an off-diagonal pair {a,b}
     plus a single m1. The 120 off-diag pairs are indexed by circular
     (distance d=1..8, anchor a) slots -> EXACTLY 128 rows, so ONE 128-deep
     matmul per atom computes T3[f,(p,m1)] (+T2[f,q] in the same pass).
   - diagonal triples {l,l,l} (and diagonal quadratic {l,l}) fold into a
     host-prepared per-element linear path over [x, x^2, x^3].
  xx pair products are built by one DVE pass over host-replicated x layouts
  (XREP0/XREP1), placing (d,a) on partitions so the matmul needs no
  transposes and no PSUM evacuations for its stationary operand.
  The finish sum_{p,m1} T3*W3*x uses host-gathered bf16 weights W3XB and one
  fused multiply-accumulate per atom, split between direct-PSUM (DVE) and
  ACT-evacuated (ACT copy + DVE) paths to balance engines.
Sharding: data-parallel over atoms, 16 atoms/core on 8 cores; per-element
weights are gathered host-side by sorted_indices (routing) and replicated.
"""

import os
import sys
from contextlib import ExitStack

import numpy as np

if "/opt/trn_rl_repo" not in sys.path:
    sys.path.insert(0, "/opt/trn_rl_repo")

B, F, L = 128, 128, 16
E = 10
P3, P2, P1 = 23, 4, 1
NCORES = 8
BS = B // NCORES  # atoms per core = 16
NPM = P3 * L  # 368 (p, m1) columns
NCOL = NPM + P2  # 372 moving columns (cubic + quadratic)

_CACHE = {}


def _bf16():
    import ml_dtypes

    return np.dtype(ml_dtypes.bfloat16)


def _build_program(na: int):
    """na = number of atoms using the ACT-evacuated finish path (0..16)."""
    import concourse.bass as bass
    import concourse.mybir as mybir
    import concourse.tile as tile
    from concourse import bacc

    dt = mybir.dt
    bf = dt.bfloat16
    f32 = dt.float32
    mult = mybir.AluOpType.mult
    add = mybir.AluOpType.add

    nc = bacc.Bacc("TRN2", target_bir_lowering=False, debug=False)

    xr0_d = nc.dram_tensor("xr0", [128, BS * 128], bf, kind="ExternalInput")
    xr1_d = nc.dram_tensor("xr1", [128, BS * 128], bf, kind="ExternalInput")
    mov_d = nc.dram_tensor("movall", [128, NCOL], bf, kind="ExternalInput")
    w3xb_d = nc.dram_tensor("w3xb", [BS, 128, NCOL], bf, kind="ExternalInput")
    xp_d = nc.dram_tensor("xp48", [128, BS * 48], bf, kind="ExternalInput")
    wu_d = nc.dram_tensor("wu48", [128, BS * 48], bf, kind="ExternalInput")
    out_d = nc.dram_tensor("outT", [128, BS], f32, kind="ExternalOutput")

    a_set = set(range(na))  # ACT-evac finish for first na atoms

    with tile.TileContext(nc) as tc, ExitStack() as ctx:
        const = ctx.enter_context(tc.tile_pool(name="const", bufs=1))
        work = ctx.enter_context(tc.tile_pool(name="work", bufs=4))
        ps = ctx.enter_context(
            tc.tile_pool(name="ps", bufs=8, space=bass.MemorySpace.PSUM)
        )

        XR0 = const.tile([128, BS * 128], bf)
        XR1 = const.tile([128, BS * 128], bf)
        MOV = const.tile([128, NCOL], bf)
        W3XB = const.tile([128, BS * NCOL], bf)
        XP = const.tile([128, BS * 48], bf)
        WU = const.tile([128, BS * 48], bf)

        nc.sync.dma_start(XR0[:], xr0_d.ap())
        nc.sync.dma_start(XR1[:], xr1_d.ap())
        nc.sync.dma_start(MOV[:], mov_d.ap())
        # w3xb in 4-atom chunks so early finishes don't wait for the tail
        for c in range(4):
            nc.sync.dma_start(
                W3XB[:, c * 4 * NCOL : (c + 1) * 4 * NCOL].rearrange(
                    "p (a x) -> p a x", a=4
                ),
                w3xb_d.ap()[c * 4 : (c + 1) * 4].rearrange("a p x -> p a x"),
            )
        nc.sync.dma_start(XP[:], xp_d.ap())
        nc.sync.dma_start(WU[:], wu_d.ap())

        # xx pair products: (d,a) on partitions, (n,f) on free dim
        XXT = const.tile([128, BS * 128], bf)
        for i in range(4):
            sl = slice(i * 512, (i + 1) * 512)
            eng = nc.vector if i < 2 else nc.gpsimd
            eng.tensor_tensor(XXT[:, sl], XR0[:, sl], XR1[:, sl], op=mult)

        ACCC = const.tile([128, BS], f32)
        for n in range(BS):
            t3 = ps.tile([128, NCOL], f32, tag="t3")
            nc.tensor.matmul(
                t3[:], XXT[:, n * 128 : (n + 1) * 128], MOV[:],
                start=True, stop=True,
            )
            wsl = W3XB[:, n * NCOL : (n + 1) * NCOL]
            if n in a_set:
                t3s = work.tile([128, NCOL], bf, tag="t3s")
                nc.scalar.copy(t3s[:], t3[:])
                scrap = work.tile([128, NCOL], bf, tag="scrap")
                nc.vector.scalar_tensor_tensor(
                    out=scrap[:], in0=t3s[:], scalar=1.0, in1=wsl,
                    op0=mult, op1=mult, accum_out=ACCC[:, n : n + 1],
                )
            else:
                scrap = work.tile([128, NCOL], f32, tag="scrapf")
                nc.vector.scalar_tensor_tensor(
                    out=scrap[:], in0=t3[:], scalar=1.0, in1=wsl,
                    op0=mult, op1=mult, accum_out=ACCC[:, n : n + 1],
                )

        # linear path: [x|x^2|x^3] * WU, grouped-reduce per atom
        LINM = const.tile([128, BS * 48], bf)
        nc.vector.tensor_tensor(LINM[:], XP[:], WU[:], op=mult)
        LINR = const.tile([128, BS], f32)
        nc.vector.tensor_reduce(
            LINR[:], LINM[:].rearrange("p (n c) -> p n c", n=BS),
            axis=mybir.AxisListType.X, op=add,
        )
        OUT = const.tile([128, BS], f32)
        nc.vector.tensor_tensor(OUT[:], ACCC[:], LINR[:], op=add)
        nc.sync.dma_start(out_d.ap(), OUT[:])

    nc.compile()
    return nc


def _pair_slots():
    """Map off-diag pair {a1,a2} -> [(row, frac)] in the (d,a) basis."""
    table = {}
    for a1 in range(L):
        for a2 in range(a1 + 1, L):
            d1 = (a2 - a1) % 16
            if 1 <= d1 <= 7:
                slots = [((d1 - 1) * 16 + a1, 1.0)]
            elif d1 == 8:
                slots = [(7 * 16 + a1, 0.5), (7 * 16 + a2, 0.5)]
            else:
                slots = [((16 - d1 - 1) * 16 + a2, 1.0)]
            table[(a1, a2)] = slots
    return table


def _build_weights(U3, U2, U1, W3, W2, W1):
    """Shared (element-indexed) weight transforms. Returns
    (MOVALL [128, 372] f32, wu48 [E, F, 48] f32)."""
    U3r = U3.reshape(P3, L, L, L).astype(np.float64)
    U2r = U2.reshape(P2, L, L).astype(np.float64)
    U1r = U1.reshape(L).astype(np.float64)
    W3_ = W3.astype(np.float64)
    W2_ = W2.astype(np.float64)
    W1_ = W1.astype(np.float64)

    C3 = (U3r + U3r.transpose(0, 1, 3, 2) + U3r.transpose(0, 2, 1, 3)
          + U3r.transpose(0, 2, 3, 1) + U3r.transpose(0, 3, 1, 2)
          + U3r.transpose(0, 3, 2, 1))

    slots = _pair_slots()
    M3 = np.zeros((128, NPM))
    c3d = np.zeros((P3, L))  # diag-triple coefficients
    for p in range(P3):
        for i in range(L):
            for j in range(i, L):
                for k in range(j, L):
                    s = {i, j, k}
                    if len(s) == 1:
                        c3d[p, i] += C3[p, i, j, k] / 6.0
                        continue
                    if len(s) == 2:
                        coef = C3[p, i, j, k] / 2.0
                        pr, single = ((i, k), i) if i == j else ((i, j), j)
                    else:
                        coef = C3[p, i, j, k]
                        pr, single = (i, j), k
                    for row, frac in slots[pr]:
                        M3[row, p * L + single] += frac * coef

    M2 = np.zeros((128, P2))
    U2s = U2r + U2r.transpose(0, 2, 1)
    for q in range(P2):
        for a1 in range(L):
            for a2 in range(a1 + 1, L):
                for row, frac in slots[(a1, a2)]:
                    M2[row, q] += frac * U2s[q, a1, a2]

    MOVALL = np.concatenate([M3, M2], axis=1).astype(np.float32)

    u2d = np.stack([U2r[:, l, l] for l in range(L)], axis=1)  # [P2, L]
    wu1 = np.einsum("ef,l->efl", W1_[:, 0, :], U1r)
    wu2d = np.einsum("eqf,ql->efl", W2_, u2d)
    wu3d = np.einsum("epf,pl->efl", W3_, c3d)
    wu48 = np.concatenate([wu1, wu2d, wu3d], axis=2).astype(np.float32)  # [E,F,48]
    return MOVALL, wu48


def _host_prep(x, sorted_indices, weights_max, w2, w1, U3, U2, U1):
    BF = _bf16()
    x = np.ascontiguousarray(x, dtype=np.float32)
    si = np.asarray(sorted_indices).astype(np.int64)
    W3 = np.asarray(weights_max, dtype=np.float32)
    W2 = np.asarray(w2, dtype=np.float32)
    W1 = np.asarray(w1, dtype=np.float32)

    MOVALL, wu48 = _build_weights(
        np.asarray(U3), np.asarray(U2), np.asarray(U1), W3, W2, W1
    )
    movb = MOVALL.astype(BF)

    # circular shift indices for XREP1: row (d,a) reads x[., (a+d+1)%16]
    d_idx = np.arange(8).repeat(16)  # d-1 per row
    a_idx = np.tile(np.arange(16), 8)
    sh_idx = (a_idx + d_idx + 1) % 16  # [128]

    in_maps = []
    for c in range(NCORES):
        sl = slice(c * BS, (c + 1) * BS)
        xc = x[sl]  # [BS, F, L]
        sic = si[sl]
        xb = xc.astype(BF).astype(np.float32)  # bf16-rounded x

        # XREP0 [128 rows (d,a), (n,f)]: x[n, f, a]
        xT = xb.transpose(2, 0, 1).reshape(L, BS * 128)  # [a, (n,f)]
        xr0 = np.ascontiguousarray(np.broadcast_to(xT[a_idx], (128, BS * 128)))
        xr1 = np.ascontiguousarray(xT[sh_idx])

        # w3xb [n, f, (p,m1)] = W3[e,p,f]*x[n,f,m1]; cols 368:372 = W2[e,q,f]
        w3g = W3[sic]  # [BS, P3, F]
        w3x = w3g.transpose(0, 2, 1)[:, :, :, None] * xc[:, :, None, :]
        w3x = w3x.reshape(BS, 128, NPM)
        w2g = W2[sic].transpose(0, 2, 1)  # [BS, F, P2]
        w3xb = np.concatenate([w3x, w2g], axis=2)  # [BS, 128, 372]

        # xp48 [f, (n, 48)] = [x | x^2 | x^3]
        xp = np.concatenate([xc, xc**2, xc**3], axis=2)  # [BS, F, 48]
        xp48 = xp.transpose(1, 0, 2).reshape(128, BS * 48)

        # wu48 gathered per atom [f, (n, 48)]
        wug = wu48[sic].transpose(1, 0, 2).reshape(128, BS * 48)

        in_maps.append(
            {
                "xr0": xr0.astype(BF),
                "xr1": xr1.astype(BF),
                "movall": movb,
                "w3xb": w3xb.astype(BF),
                "xp48": xp48.astype(BF),
                "wu48": wug.astype(BF),
            }
        )
    return in_maps


def _get_nc():
    na = int(os.environ.get("KERNEL_NA", "16"))
    key = ("nc", na)
    if key not in _CACHE:
        _CACHE[key] = _build_program(na)
    return _CACHE[key]


def kernel(
    x,
    bincount,
    sorted_indices,
    weights_max,
    w2,
    w1,
    U3,
    U2,
    U1,
    _trace=False,
):
    from concourse.bass_utils import run_bass_kernel_spmd

    nc = _get_nc()
    in_maps = _host_prep(x, sorted_indices, weights_max, w2, w1, U3, U2, U1)
    res = run_bass_kernel_spmd(
        nc, in_maps, core_ids=list(range(NCORES)), trace=_trace
    )
    outs = [res.results[c]["outT"] for c in range(NCORES)]  # each [128f, 16n]
    full = np.concatenate([np.asarray(o, dtype=np.float32).T for o in outs], axis=0)
    out = np.ascontiguousarray(full, dtype=np.float32)
    if _trace:
        return out, res
    return out
